# revision 7
# baseline (speedup 1.0000x reference)
"""Bass/Trainium2 kernel for a 3-layer GCN over a batch of graphs.

Strategy (data-parallel, one graph per NeuronCore):
  - Host: sort each graph's edges by destination (order-only transform; the
    segment-sum is order-invariant), bucket them into 157 destination windows
    of 128 nodes, pad each window to a fixed 2432 edge slots so that the
    device program is fully static and shared by all 8 cores (SPMD).
  - Device, per layer (aggregation done on the narrow side of each GEMM):
      h~ rows live in DRAM node-major bf16; dma_gather pulls h~[src] for a
      window's edges into SBUF edge-major tiles; per-edge weights are applied
      by the Scalar engine (Copy activation with a per-partition scale); a
      plain one-hot matrix (iota == dst_local, one bf16 DVE op per 128-edge
      chunk) feeds the tensor engine, which performs the scatter-add as a
      PSUM-accumulated matmul chain.  Degrees use the same one-hots with the
      bf16 edge-weight column as the moving operand.  Per-node work (rsqrt
      scaling, GEMMs, bias, relu) is O(N*width) in fp32 on PE/ACT/DVE.
"""

import os
import numpy as np

import concourse.bacc as bacc
import concourse.bass as bass
import concourse.mybir as mybir
from concourse import tile
from concourse.bass_utils import run_bass_kernel_spmd

G, N, E = 8, 20000, 320000
STATE, HID, EMB, POS, DEPTH = 64, 128, 64, 16, 4
NW = (N + 127) // 128          # 157 destination windows of 128 nodes
CH = 19                        # 128-edge chunks per window (mean 16 + 8.5 sigma)
SLOTS = CH * 128               # 2432 padded edge slots per window
PTOT = NW * SLOTS              # total padded slots
NPAD = NW * 128                # 20096 padded node rows in scratch DRAM
GRP = 2                        # windows per dma_gather call

F32 = mybir.dt.float32
BF16 = mybir.dt.bfloat16
I16 = mybir.dt.int16
I32 = mybir.dt.int32
OP = mybir.AluOpType
AF = mybir.ActivationFunctionType

_NC_CACHE = {}
LAST_RESULTS = None  # BassKernelResults of the most recent run (for test harness)


def build_nc():
    nc = bacc.Bacc(None)

    x_in = nc.dram_tensor("x", [N, STATE], F32, kind="ExternalInput")
    srcidx = nc.dram_tensor("srcidx", [128, PTOT // 16], I16, kind="ExternalInput")
    dstl = nc.dram_tensor("dstl", [128, PTOT // 128], F32, kind="ExternalInput")
    ewt = nc.dram_tensor("ew", [128, PTOT // 128], F32, kind="ExternalInput")
    posi = nc.dram_tensor("posi", [128, 8], I16, kind="ExternalInput")
    w0 = nc.dram_tensor("W0", [STATE, HID], F32, kind="ExternalInput")
    w1 = nc.dram_tensor("W1", [HID, HID], F32, kind="ExternalInput")
    w2 = nc.dram_tensor("W2", [HID, EMB], F32, kind="ExternalInput")
    b0 = nc.dram_tensor("b0", [128, HID], F32, kind="ExternalInput")
    b1 = nc.dram_tensor("b1", [128, HID], F32, kind="ExternalInput")
    b2 = nc.dram_tensor("b2", [128, EMB], F32, kind="ExternalInput")
    out = nc.dram_tensor("out", [POS, EMB], F32, kind="ExternalOutput")

    # gather tables: bf16, padded to 128 features (gather elem must be a
    # multiple of 256 bytes; unused columns are never consumed by the PE)
    xt_d = nc.dram_tensor("xt_d", [NPAD, 128], BF16)
    h1_d = nc.dram_tensor("h1_d", [NPAD, 128], BF16)
    t2_d = nc.dram_tensor("t2_d", [NPAD, 128], BF16)
    emb_d = nc.dram_tensor("emb_d", [NPAD, EMB], F32)

    ICOLS = PTOT // 16   # srcidx columns
    MCOLS = PTOT // 128  # dstl/ew columns
    IW = SLOTS // 16     # srcidx columns per window
    # gather call groups: [(first_window, n_windows), ...]
    groups = [(w, min(GRP, NW - w)) for w in range(0, NW, GRP)]

    with tile.TileContext(nc) as tc:
        with (
            tc.tile_pool(name="const", bufs=1) as cpool,
            tc.tile_pool(name="meta", bufs=1) as mpool,
            tc.tile_pool(name="work", bufs=3) as wpool,
            tc.tile_pool(name="node", bufs=3) as npool,
            tc.tile_pool(name="opool", bufs=6) as opool,
            tc.tile_pool(name="psS", bufs=2, space="PSUM") as psS,
            tc.tile_pool(name="psT", bufs=2, space="PSUM") as psT,
            tc.tile_pool(name="psZ", bufs=2, space="PSUM") as psZ,
            tc.tile_pool(name="psD", bufs=2, space="PSUM") as psD,
        ):
            # ---- constants -------------------------------------------------
            iota_i = cpool.tile([128, 128], I32, tag="ioi")
            nc.gpsimd.iota(iota_i[:], [[1, 128]], base=0, channel_multiplier=0)
            iota_b = cpool.tile([128, 128], BF16, tag="iob")
            nc.vector.tensor_copy(iota_b[:], iota_i[:])
            iota_f = cpool.tile([128, 128], F32, tag="iof")
            nc.vector.tensor_copy(iota_f[:], iota_i[:])
            pidx_i = cpool.tile([128, 1], I32, tag="pii")
            nc.gpsimd.iota(pidx_i[:], [[1, 1]], base=0, channel_multiplier=1)
            pidx_f = cpool.tile([128, 1], F32, tag="pif")
            nc.vector.tensor_copy(pidx_f[:], pidx_i[:])
            ident = cpool.tile([128, 128], F32, tag="ident")
            nc.vector.tensor_scalar(ident[:], iota_f[:], pidx_f[:], None, OP.is_equal)

            w0_t = cpool.tile([STATE, HID], F32, tag="w0")
            nc.sync.dma_start(w0_t[:], w0[:])
            w1_t = cpool.tile([HID, HID], F32, tag="w1")
            nc.sync.dma_start(w1_t[:], w1[:])
            w2_t = cpool.tile([HID, EMB], F32, tag="w2")
            nc.sync.dma_start(w2_t[:], w2[:])
            b0_t = cpool.tile([128, HID], F32, tag="b0")
            nc.sync.dma_start(b0_t[:], b0[:])
            b1_t = cpool.tile([128, HID], F32, tag="b1")
            nc.sync.dma_start(b1_t[:], b1[:])
            b2_t = cpool.tile([128, EMB], F32, tag="b2")
            nc.sync.dma_start(b2_t[:], b2[:])

            # ---- resident edge metadata -----------------------------------
            src_t = mpool.tile([128, ICOLS], I16, tag="srcidx")
            nc.sync.dma_start(src_t[:], srcidx[:])
            dstl_t = mpool.tile([128, MCOLS], F32, tag="dstl")
            nc.sync.dma_start(dstl_t[:], dstl[:])
            ew_t = mpool.tile([128, MCOLS], F32, tag="ew")
            nc.sync.dma_start(ew_t[:], ewt[:])
            ewb_t = mpool.tile([128, MCOLS], BF16, tag="ewb")
            nc.vector.tensor_copy(ewb_t[:], ew_t[:])
            posi_t = mpool.tile([128, 8], I16, tag="posi")
            nc.sync.dma_start(posi_t[:], posi[:])

            dinv_t = cpool.tile([128, NW], F32, tag="dinv")

            def onehot(k_col):
                """[128 edges, 128 dst] bf16 one-hot (no weight)."""
                o = opool.tile([128, 128], BF16, tag="O")
                nc.vector.tensor_scalar(
                    o[:], iota_b[:], dstl_t[:, k_col : k_col + 1], None, OP.is_equal
                )
                return o

            # ---- degrees + dinv + x~ --------------------------------------
            for w in range(NW):
                deg = psD.tile([128, 1], F32, tag="deg")
                for k in range(CH):
                    col = w * CH + k
                    o = onehot(col)
                    nc.tensor.matmul(
                        deg[:], o[:], ewb_t[:, col : col + 1],
                        start=(k == 0), stop=(k == CH - 1),
                    )
                sq = npool.tile([128, 1], F32, tag="sq")
                nc.scalar.activation(sq[:], deg[:], AF.Sqrt, bias=1.0)
                nc.vector.reciprocal(dinv_t[:, w : w + 1], sq[:])

                xt = npool.tile([128, STATE], F32, tag="xt")
                lo = w * 128
                if lo + 128 <= N:
                    nc.sync.dma_start(xt[:], x_in[lo : lo + 128, :])
                    nc.vector.tensor_scalar_mul(xt[:], xt[:], dinv_t[:, w : w + 1])
                else:
                    nt = N - lo
                    nc.vector.memset(xt[:], 0.0)
                    nc.sync.dma_start(xt[:nt, :], x_in[lo:N, :])
                    nc.vector.tensor_scalar_mul(
                        xt[:nt, :], xt[:nt, :], dinv_t[:nt, w : w + 1]
                    )
                xtb = npool.tile([128, STATE], BF16, tag="xtb")
                nc.vector.tensor_copy(xtb[:], xt[:])
                nc.sync.dma_start(xt_d[lo : lo + 128, :STATE], xtb[:])

            # ---- layer machinery ------------------------------------------
            def gather_group(wg, nwin, src_d):
                msgs = wpool.tile([128, GRP * CH, 128], BF16, tag="msgs")
                nidx = nwin * SLOTS
                nc.gpsimd.dma_gather(
                    msgs[:, : nwin * CH, :], src_d[:],
                    src_t[:, wg * IW : wg * IW + nwin * IW],
                    nidx, nidx, 128, single_packet=False,
                )
                return msgs

            def scatter_window(w, msgs, coff, width):
                """msgs chunk columns coff.. hold this window's edges."""
                s = psS.tile([128, width], F32, tag="S")
                for k in range(CH):
                    col = w * CH + k
                    # apply edge weights on ACT: in-place scaled copy
                    mk = msgs[:, coff + k, :width]
                    nc.scalar.activation(
                        mk, mk, AF.Copy, scale=ew_t[:, col : col + 1]
                    )
                    o = onehot(col)
                    nc.tensor.matmul(
                        s[:], o[:], mk, start=(k == 0), stop=(k == CH - 1)
                    )
                return s

            def gemm(u, width, wt, wout):
                """node-major u [128, width] f32 -> z_psum [128, wout] = u @ Wt"""
                ut_ps = psT.tile([128, 128], F32, tag="T")
                nc.tensor.transpose(ut_ps[:width, :], u[:], ident[:])
                ut = npool.tile([128, 128], F32, tag="uT")
                nc.scalar.copy(ut[:width, :], ut_ps[:width, :])
                z_ps = psZ.tile([128, HID], F32, tag="Z")
                nc.tensor.matmul(z_ps[:, :wout], ut[:width, :], wt[:])
                return z_ps

            def self_tile(src_d, lo, width):
                """load h~ tile back (bf16) and widen to f32"""
                hb = npool.tile([128, width], BF16, tag="hb")
                nc.sync.dma_start(hb[:], src_d[lo : lo + 128, :width])
                hf = npool.tile([128, width], F32, tag="hf")
                nc.vector.tensor_copy(hf[:], hb[:])
                return hf

            # L0: aggregate x~ (w=64); z = dinv*(S+x~) @ W0 + b0; h1~ -> dram
            for wg, nwin in groups:
                msgs = gather_group(wg, nwin, xt_d)
                for j in range(nwin):
                    w = wg + j
                    lo = w * 128
                    s = scatter_window(w, msgs, j * CH, STATE)
                    xt = self_tile(xt_d, lo, STATE)
                    a = npool.tile([128, STATE], F32, tag="a0")
                    nc.vector.tensor_add(a[:], s[:], xt[:])
                    nc.vector.tensor_scalar_mul(a[:], a[:], dinv_t[:, w : w + 1])
                    z_ps = gemm(a, STATE, w0_t, HID)
                    zb = npool.tile([128, HID], F32, tag="zb")
                    nc.vector.tensor_add(zb[:], z_ps[:], b0_t[:])
                    h = npool.tile([128, HID], F32, tag="h")
                    nc.scalar.activation(h[:], zb[:], AF.Relu)
                    nc.vector.tensor_scalar_mul(h[:], h[:], dinv_t[:, w : w + 1])
                    hbo = npool.tile([128, HID], BF16, tag="hbo")
                    nc.vector.tensor_copy(hbo[:], h[:])
                    nc.sync.dma_start(h1_d[lo : lo + 128, :], hbo[:])

            # L1: aggregate h1~ (w=128); h2 = relu(z); t~ = dinv*(h2@W2) -> dram
            for wg, nwin in groups:
                msgs = gather_group(wg, nwin, h1_d)
                for j in range(nwin):
                    w = wg + j
                    lo = w * 128
                    s = scatter_window(w, msgs, j * CH, HID)
                    ht = self_tile(h1_d, lo, HID)
                    a = npool.tile([128, HID], F32, tag="a1")
                    nc.vector.tensor_add(a[:], s[:], ht[:])
                    nc.vector.tensor_scalar_mul(a[:], a[:], dinv_t[:, w : w + 1])
                    z_ps = gemm(a, HID, w1_t, HID)
                    zb = npool.tile([128, HID], F32, tag="zb")
                    nc.vector.tensor_add(zb[:], z_ps[:], b1_t[:])
                    h2 = npool.tile([128, HID], F32, tag="h")
                    nc.scalar.activation(h2[:], zb[:], AF.Relu)
                    t_ps = gemm(h2, HID, w2_t, EMB)
                    tt = npool.tile([128, EMB], F32, tag="tt")
                    nc.vector.tensor_scalar_mul(
                        tt[:], t_ps[:, :EMB], dinv_t[:, w : w + 1]
                    )
                    tb = npool.tile([128, EMB], BF16, tag="tb")
                    nc.vector.tensor_copy(tb[:], tt[:])
                    nc.sync.dma_start(t2_d[lo : lo + 128, :EMB], tb[:])

            # L2: aggregate t~ (w=64); emb = dinv*(S + t~) + b2
            for wg, nwin in groups:
                msgs = gather_group(wg, nwin, t2_d)
                for j in range(nwin):
                    w = wg + j
                    lo = w * 128
                    s = scatter_window(w, msgs, j * CH, EMB)
                    tt = self_tile(t2_d, lo, EMB)
                    a = npool.tile([128, EMB], F32, tag="a2")
                    nc.vector.tensor_add(a[:], s[:], tt[:])
                    nc.vector.tensor_scalar_mul(a[:], a[:], dinv_t[:, w : w + 1])
                    e = npool.tile([128, EMB], F32, tag="e")
                    nc.vector.tensor_add(e[:], a[:], b2_t[:, :EMB])
                    nc.sync.dma_start(emb_d[lo : lo + 128, :], e[:])

            # ---- final: out = emb[pos] ------------------------------------
            pg = wpool.tile([128, 1, EMB], F32, tag="pg")
            nc.gpsimd.dma_gather(pg[:], emb_d[:], posi_t[:], 128, 128, EMB)
            nc.sync.dma_start(out[:], pg[:POS, 0, :])

    nc.compile()
    return nc


def _get_nc():
    if "nc" not in _NC_CACHE:
        _NC_CACHE["nc"] = build_nc()
    return _NC_CACHE["nc"]


def prep_core_inputs(xg, eig, ewg, posg, W0, b0, W1, b1, W2, b2):
    src = np.asarray(eig[0], np.int64)
    dst = np.asarray(eig[1], np.int64)
    ew = np.asarray(ewg, np.float32)

    order = np.argsort(dst, kind="stable")
    src_s, dst_s, ew_s = src[order], dst[order], ew[order]
    win = dst_s >> 7
    starts = np.searchsorted(win, np.arange(NW))
    cnt = np.diff(np.append(starts, E))
    assert cnt.max() <= SLOTS, f"window overflow: {cnt.max()} > {SLOTS}"
    slot = win * SLOTS + (np.arange(E) - starts[win])

    s_src = np.zeros(PTOT, np.int16)
    s_dstl = np.full(PTOT, -1.0, np.float32)
    s_ew = np.zeros(PTOT, np.float32)
    s_src[slot] = src_s.astype(np.int16)
    s_dstl[slot] = (dst_s & 127).astype(np.float32)
    s_ew[slot] = ew_s

    posp = np.zeros(128, np.int16)
    posp[:POS] = np.maximum(np.asarray(posg, np.int64), 0).astype(np.int16)

    return {
        "x": np.ascontiguousarray(xg, np.float32),
        "srcidx": np.ascontiguousarray(np.tile(s_src.reshape(PTOT // 16, 16).T, (8, 1))),
        "dstl": np.ascontiguousarray(s_dstl.reshape(PTOT // 128, 128).T),
        "ew": np.ascontiguousarray(s_ew.reshape(PTOT // 128, 128).T),
        "posi": np.ascontiguousarray(np.tile(posp.reshape(8, 16).T, (8, 1))),
        "W0": np.ascontiguousarray(W0, np.float32),
        "W1": np.ascontiguousarray(W1, np.float32),
        "W2": np.ascontiguousarray(W2, np.float32),
        "b0": np.ascontiguousarray(np.tile(np.asarray(b0, np.float32)[None, :], (128, 1))),
        "b1": np.ascontiguousarray(np.tile(np.asarray(b1, np.float32)[None, :], (128, 1))),
        "b2": np.ascontiguousarray(np.tile(np.asarray(b2, np.float32)[None, :], (128, 1))),
    }


def kernel(x, edge_index, edge_weight, pos, W0, b0, W1, b1, W2, b2):
    global LAST_RESULTS
    nc = _get_nc()
    x = np.asarray(x)
    edge_index = np.asarray(edge_index)
    edge_weight = np.asarray(edge_weight)
    pos = np.asarray(pos)
    in_maps = [
        prep_core_inputs(
            x[g], edge_index[g], edge_weight[g], pos[g], W0, b0, W1, b1, W2, b2
        )
        for g in range(G)
    ]
    trace = os.environ.get("GNN_BASS_TRACE", "0") not in ("", "0")
    res = run_bass_kernel_spmd(
        nc, in_maps, core_ids=list(range(G)), trace=trace,
        trace_cores=list(range(G)) if trace else None,
    )
    LAST_RESULTS = res
    outs = []
    for g in range(G):
        og = res.results[g]["out"].astype(np.float32)
        og = np.where(np.asarray(pos[g])[:, None] != -1, og, np.float32(-DEPTH))
        outs.append(og.reshape(POS * EMB))
    return np.stack(outs).astype(np.float32)
